# revision 4
# baseline (speedup 1.0000x reference)
"""Trainium2 Bass kernel for the GCM (global context module) problem.

Computation per batch sample b (x_b = x[b] viewed as [C=512, HW=9216]):
    x1 = w1 @ x_b                      [128, HW]
    x2 = w2 @ x_b                      [256, HW]
    v  = softmax_all(x1 @ x2^T)        [128, 256]  (softmax over all 32768)
    n  = relu(v + w3 @ v)              [128, 256]
    z  = w4 @ n^T                      [256, 128]
    W  = w5 @ z                        [512, 128]  (collapses y/conv5: w5@(z@x1) == (w5@z)@x1)
    out = x_b + W @ x1                 [512, HW]

Sharding: data-parallel over batch, one sample per NeuronCore (8 cores).

On-chip strategy per core:
  Phase 1: stream x in ([128,1536] tiles, resident in SBUF), compute
    hw-major [x1T|x2T] tiles via matmul with the X-slice as the stationary
    operand (out[hw,384] = X_slice.T @ [w1T|w2T]), accumulate
    v = x1T.T @ x2T in a persistent PSUM bank over 72 subtiles.
  Softmax: global max/sum via DVE free-dim reduce + GPSIMD partition
    all-reduce; exp on ScalarE with -max bias; normalize by 1/sum.
  Small chain: conv3+relu, PE transposes for n^T, z, W^T = z^T @ w5^T.
  Phase 2: per 512-wide tile recompute x1 (k-major) from resident x,
    x_res = W @ x1 via W^T slices as stationary, residual add on DVE
    (exact f32 read of resident x), DMA out.

All matmuls run as float32r (full PE rate at N>=256) on f32 bits; the
residual add is exact f32 (f32r storage bitcast back to f32 — same bits).
Numerically safe: the softmax here is a near-argmax (gaps >> f32r
rounding) and |x_res| << |x|.
"""

import numpy as np

import concourse.bass as bass
import concourse.tile as tile
from concourse import bacc, mybir, bass_isa
from concourse.bass_utils import run_bass_kernel_spmd
from concourse.masks import make_identity

F32 = mybir.dt.float32
F32R = mybir.dt.float32r
AX = mybir.AxisListType
AL = mybir.AluOpType
AF = mybir.ActivationFunctionType

N_CORES = 8
C = 512
H = W_IMG = 96
HW = H * W_IMG          # 9216
CK = C // 128           # 4 chunks of channels
NBLK = 6                # x blocks along hw
BLK = HW // NBLK        # 1536
NSUB = HW // 128        # 72 phase-1 subtiles
SUB_PER_BLK = BLK // 128
NT = HW // 512          # 18 phase-2 tiles
T_PER_BLK = BLK // 512
C4 = C // 4             # 128
C2 = C // 2             # 256
KM = C4 + C2            # 384 = concat(x1T, x2T) free size


def _emit(ctx, tc, aps, use_bias):
    nc = tc.nc
    x_d = aps["x"]
    w12t_d = aps["w12t"]
    w3t_d = aps["w3t"]
    w4t_d = aps["w4t"]
    w5t_d = aps["w5t"]
    out_d = aps["out"]

    consts = ctx.enter_context(tc.tile_pool(name="consts", bufs=1))

    # ---- weights to SBUF (f32r: consumed only by matmuls) ----
    w12 = []
    for c in range(CK):
        t = consts.tile([128, KM], F32R, tag=f"w12_{c}")
        nc.sync.dma_start(out=t[:], in_=w12t_d[c * 128 : (c + 1) * 128, :])
        w12.append(t)
    w3t = consts.tile([128, 128], F32R, tag="w3t")
    nc.sync.dma_start(out=w3t[:], in_=w3t_d[:, :])
    w4t = []
    for q in range(2):
        t = consts.tile([128, C2], F32R, tag=f"w4t_{q}")
        nc.sync.dma_start(out=t[:], in_=w4t_d[q * 128 : (q + 1) * 128, :])
        w4t.append(t)
    w5t = []
    for q in range(2):
        t = consts.tile([128, C], F32R, tag=f"w5t_{q}")
        nc.sync.dma_start(out=t[:], in_=w5t_d[q * 128 : (q + 1) * 128, :])
        w5t.append(t)
    ident = consts.tile([128, 128], F32, tag="ident")
    make_identity(nc, ident[:])

    bias_t = {}
    if use_bias:
        b12row_d = aps["b12row"]
        b1_d, b3_d, b4_d, b5_d = aps["b1c"], aps["b3c"], aps["b4c"], aps["b5c"]
        # [b1|b2] replicated across partitions, added to the hw-major tiles
        brow1 = consts.tile([1, KM], F32, tag="brow1")
        nc.sync.dma_start(out=brow1[:], in_=b12row_d[:, :])
        brow = consts.tile([128, KM], F32, tag="brow")
        nc.gpsimd.partition_broadcast(brow[:], brow1[:])
        bias_t["brow"] = brow
        b1 = consts.tile([128, 1], F32, tag="b1")
        nc.sync.dma_start(out=b1[:], in_=b1_d[:, :])
        bias_t["b1"] = b1
        b3 = consts.tile([128, 1], F32, tag="b3")
        nc.sync.dma_start(out=b3[:], in_=b3_d[:, :])
        bias_t["b3"] = b3
        b4 = []
        for q in range(2):
            t = consts.tile([128, 1], F32, tag=f"b4_{q}")
            nc.sync.dma_start(out=t[:], in_=b4_d[q * 128 : (q + 1) * 128, :])
            b4.append(t)
        bias_t["b4"] = b4
        b5 = []
        for oc in range(CK):
            t = consts.tile([128, 1], F32, tag=f"b5_{oc}")
            nc.sync.dma_start(out=t[:], in_=b5_d[oc * 128 : (oc + 1) * 128, :])
            b5.append(t)
        bias_t["b5"] = b5

    # ---- x resident in SBUF: 24 tiles [128, 1536] f32r ----
    xpool = ctx.enter_context(tc.tile_pool(name="x", bufs=1))
    xt = {}
    for b in range(NBLK):
        for c in range(CK):
            t = xpool.tile([128, BLK], F32R, tag=f"x_{c}_{b}")
            nc.sync.dma_start(
                out=t[:],
                in_=x_d[c * 128 : (c + 1) * 128, b * BLK : (b + 1) * BLK],
            )
            xt[(c, b)] = t

    sm = ctx.enter_context(tc.tile_pool(name="sm", bufs=1))

    # ---- phase 1: x12T tiles + v accumulation ----
    with (
        tc.tile_pool(name="psA", bufs=3, space="PSUM") as psA,
        tc.tile_pool(name="vps", bufs=1, space="PSUM") as vps,
        tc.tile_pool(name="xtp", bufs=4) as xtp,
    ):
        v_ps = vps.tile([128, C2], F32, tag="v")

        def emit_v(s, xtile):
            nc.tensor.matmul(
                v_ps[:],
                xtile[:, 0:C4],
                xtile[:, C4:KM],
                start=(s == 0),
                stop=(s == NSUB - 1),
            )

        SKEW = 2
        pend = []
        for s in range(NSUB):
            b, off = divmod(s, SUB_PER_BLK)
            off *= 128
            ps = psA.tile([128, KM], F32, tag="xts")
            for c in range(CK):
                nc.tensor.matmul(
                    ps[:],
                    xt[(c, b)][:, off : off + 128],
                    w12[c][:],
                    start=(c == 0),
                    stop=(c == CK - 1),
                )
            xtile = xtp.tile([128, KM], F32R, tag="xt")
            if use_bias:
                nc.vector.tensor_tensor(
                    xtile[:], ps[:], bias_t["brow"][:], op=AL.add
                )
            else:
                nc.scalar.copy(xtile[:], ps[:])
            pend.append((s, xtile))
            if len(pend) > SKEW:
                emit_v(*pend.pop(0))
        while pend:
            emit_v(*pend.pop(0))

        # ---- softmax over all 32768 entries of v ----
        m1 = sm.tile([128, 1], F32, tag="m1")
        nc.vector.tensor_reduce(m1[:], v_ps[:], axis=AX.X, op=AL.max)
        mall = sm.tile([128, 1], F32, tag="mall")
        nc.gpsimd.partition_all_reduce(mall[:], m1[:], 128, bass_isa.ReduceOp.max)
        negm = sm.tile([128, 1], F32, tag="negm")
        nc.vector.tensor_scalar_mul(negm[:], mall[:], -1.0)
        e = sm.tile([128, C2], F32, tag="e")
        nc.scalar.activation(e[:], v_ps[:], AF.Exp, bias=negm[:], scale=1.0)

    s1 = sm.tile([128, 1], F32, tag="s1")
    nc.vector.tensor_reduce(s1[:], e[:], axis=AX.X, op=AL.add)
    sall = sm.tile([128, 1], F32, tag="sall")
    nc.gpsimd.partition_all_reduce(sall[:], s1[:], 128, bass_isa.ReduceOp.add)
    sinv = sm.tile([128, 1], F32, tag="sinv")
    nc.vector.reciprocal(sinv[:], sall[:])
    en = sm.tile([128, C2], F32R, tag="en")
    nc.vector.tensor_scalar_mul(en[:], e[:], sinv[:])

    # ---- small chain: conv3+relu, n^T, z, W^T ----
    wt = sm.tile([128, C], F32R, tag="wt")
    with tc.tile_pool(name="psB", bufs=2, space="PSUM") as psB:
        ps3 = psB.tile([128, C2], F32, tag="ps3")
        nc.tensor.matmul(ps3[:], w3t[:], en[:], start=True, stop=True)
        nsb = sm.tile([128, C2], F32, tag="nsb")
        b3s = bias_t["b3"][:] if use_bias else 0.0
        nc.vector.scalar_tensor_tensor(
            nsb[:], ps3[:], b3s, en[:].bitcast(F32), op0=AL.add, op1=AL.add
        )
        nc.vector.tensor_scalar_max(nsb[:], nsb[:], 0.0)

        nts = []
        for q in range(2):
            pT = psB.tile([128, 128], F32, tag="pT")
            nc.tensor.transpose(pT[:], nsb[:, q * 128 : (q + 1) * 128], ident[:])
            ntq = sm.tile([128, 128], F32R, tag=f"nt{q}")
            nc.scalar.copy(ntq[:], pT[:])
            nts.append(ntq)

        zs = []
        for mc in range(2):
            pz = psB.tile([128, 128], F32, tag="pz")
            for q in range(2):
                nc.tensor.matmul(
                    pz[:],
                    w4t[q][:, mc * 128 : (mc + 1) * 128],
                    nts[q][:],
                    start=(q == 0),
                    stop=(q == 1),
                )
            zq = sm.tile([128, 128], F32R, tag=f"z{mc}")
            if use_bias:
                nc.scalar.add(zq[:], pz[:], bias_t["b4"][mc][:])
            else:
                nc.scalar.copy(zq[:], pz[:])
            zs.append(zq)

        pW = psB.tile([128, C], F32, tag="pW")
        for mc in range(2):
            nc.tensor.matmul(
                pW[:], zs[mc][:], w5t[mc][:], start=(mc == 0), stop=(mc == 1)
            )
        nc.scalar.copy(wt[:], pW[:])

    # ---- phase 2: x1 recompute, x_res = W @ x1, residual add, DMA out ----
    with (
        tc.tile_pool(name="psC", bufs=2, space="PSUM") as psC,
        tc.tile_pool(name="psD", bufs=4, space="PSUM") as psD,
        tc.tile_pool(name="x1p", bufs=3) as x1p,
        tc.tile_pool(name="outp", bufs=6) as outp,
    ):

        def emit_tail(t, x1tile):
            b, off = divmod(t, T_PER_BLK)
            off *= 512
            goff = t * 512
            for oc in range(CK):
                pr = psD.tile([128, 512], F32, tag="pr")
                nc.tensor.matmul(
                    pr[:],
                    wt[:, oc * 128 : (oc + 1) * 128],
                    x1tile[:],
                    start=True,
                    stop=True,
                )
                ot = outp.tile([128, 512], F32, tag="ot")
                xs = xt[(oc, b)][:, off : off + 512].bitcast(F32)
                if use_bias:
                    nc.vector.scalar_tensor_tensor(
                        ot[:], pr[:], bias_t["b5"][oc][:], xs, op0=AL.add, op1=AL.add
                    )
                else:
                    nc.vector.tensor_tensor(ot[:], pr[:], xs, op=AL.add)
                nc.sync.dma_start(
                    out=out_d[oc * 128 : (oc + 1) * 128, goff : goff + 512],
                    in_=ot[:],
                )

        prev = None
        for t in range(NT):
            b, off = divmod(t, T_PER_BLK)
            off *= 512
            px1 = psC.tile([128, 512], F32, tag="px1")
            for c in range(CK):
                nc.tensor.matmul(
                    px1[:],
                    w12[c][:, 0:C4],
                    xt[(c, b)][:, off : off + 512],
                    start=(c == 0),
                    stop=(c == CK - 1),
                )
            x1tile = x1p.tile([128, 512], F32R, tag="x1t")
            if use_bias:
                nc.scalar.add(x1tile[:], px1[:], bias_t["b1"][:])
            else:
                nc.scalar.copy(x1tile[:], px1[:])
            if prev is not None:
                emit_tail(*prev)
            prev = (t, x1tile)
        emit_tail(*prev)


def _build(use_bias):
    nc = bacc.Bacc("TRN2", target_bir_lowering=False, debug=False, num_devices=N_CORES)
    aps = {
        "x": nc.dram_tensor("x", [C, HW], F32R, kind="ExternalInput").ap(),
        "w12t": nc.dram_tensor("w12t", [C, KM], F32R, kind="ExternalInput").ap(),
        "w3t": nc.dram_tensor("w3t", [C4, C4], F32R, kind="ExternalInput").ap(),
        "w4t": nc.dram_tensor("w4t", [C2, C2], F32R, kind="ExternalInput").ap(),
        "w5t": nc.dram_tensor("w5t", [C2, C], F32R, kind="ExternalInput").ap(),
        "out": nc.dram_tensor("out", [C, HW], F32, kind="ExternalOutput").ap(),
    }
    if use_bias:
        aps["b12row"] = nc.dram_tensor(
            "b12row", [1, KM], F32, kind="ExternalInput"
        ).ap()
        aps["b1c"] = nc.dram_tensor("b1c", [C4, 1], F32, kind="ExternalInput").ap()
        aps["b3c"] = nc.dram_tensor("b3c", [C4, 1], F32, kind="ExternalInput").ap()
        aps["b4c"] = nc.dram_tensor("b4c", [C2, 1], F32, kind="ExternalInput").ap()
        aps["b5c"] = nc.dram_tensor("b5c", [C, 1], F32, kind="ExternalInput").ap()

    from contextlib import ExitStack

    with tile.TileContext(nc) as tc:
        with ExitStack() as ctx:
            _emit(ctx, tc, aps, use_bias)
    nc.compile()
    return nc


_CACHE = {}


def _run(inputs, trace=False, **run_kwargs):
    x = np.ascontiguousarray(np.asarray(inputs["x"], dtype=np.float32))
    assert x.shape == (N_CORES, C, H, W_IMG), x.shape
    w1 = np.asarray(inputs["w1"], dtype=np.float32)
    w2 = np.asarray(inputs["w2"], dtype=np.float32)
    w3 = np.asarray(inputs["w3"], dtype=np.float32)
    w4 = np.asarray(inputs["w4"], dtype=np.float32)
    w5 = np.asarray(inputs["w5"], dtype=np.float32)
    b1 = np.asarray(inputs["b1"], dtype=np.float32)
    b2 = np.asarray(inputs["b2"], dtype=np.float32)
    b3 = np.asarray(inputs["b3"], dtype=np.float32)
    b4 = np.asarray(inputs["b4"], dtype=np.float32)
    b5 = np.asarray(inputs["b5"], dtype=np.float32)
    use_bias = bool(
        np.any(b1) or np.any(b2) or np.any(b3) or np.any(b4) or np.any(b5)
    )

    if use_bias not in _CACHE:
        _CACHE[use_bias] = _build(use_bias)
    nc = _CACHE[use_bias]

    w12t = np.ascontiguousarray(
        np.concatenate([w1.T, w2.T], axis=1), dtype=np.float32
    )  # [512, 384]
    w3t = np.ascontiguousarray(w3.T)
    w4t = np.ascontiguousarray(w4.T)
    w5t = np.ascontiguousarray(w5.T)

    shared = {"w12t": w12t, "w3t": w3t, "w4t": w4t, "w5t": w5t}
    if use_bias:
        shared["b12row"] = np.ascontiguousarray(
            np.concatenate([b1, b2])[None, :], dtype=np.float32
        )
        shared["b1c"] = np.ascontiguousarray(b1[:, None])
        shared["b3c"] = np.ascontiguousarray(b3[:, None])
        shared["b4c"] = np.ascontiguousarray(b4[:, None])
        shared["b5c"] = np.ascontiguousarray(b5[:, None])

    in_maps = [
        {"x": np.ascontiguousarray(x[b].reshape(C, HW)), **shared}
        for b in range(N_CORES)
    ]
    res = run_bass_kernel_spmd(
        nc, in_maps, core_ids=list(range(N_CORES)), trace=trace, **run_kwargs
    )
    out = np.stack(
        [res.results[b]["out"].reshape(C, H, W_IMG) for b in range(N_CORES)]
    ).astype(np.float32)
    return out, res


def kernel(**inputs):
    out, _ = _run(inputs, trace=False)
    return out


# revision 9
# speedup vs baseline: 1.0435x; 1.0435x over previous
"""Trainium2 Bass kernel for the GCM (global context module) problem.

Computation per batch sample b (x_b = x[b] viewed as [C=512, HW=9216]):
    x1 = w1 @ x_b                      [128, HW]
    x2 = w2 @ x_b                      [256, HW]
    v  = softmax_all(x1 @ x2^T)        [128, 256]  (softmax over all 32768)
    n  = relu(v + w3 @ v)              [128, 256]
    z  = w4 @ n^T                      [256, 128]
    W  = w5 @ z                        [512, 128]  (collapses y/conv5: w5@(z@x1) == (w5@z)@x1)
    out = x_b + W @ x1                 [512, HW]

Sharding: data-parallel over batch, one sample per NeuronCore (8 cores).

On-chip strategy per core:
  Phase 1: stream x in ([128,1536] tiles, resident in SBUF), compute
    hw-major [x1T|x2T] tiles via matmul with the X-slice as the stationary
    operand (out[hw,384] = X_slice.T @ [w1T|w2T]), accumulate
    v = x1T.T @ x2T in a persistent PSUM bank over 72 subtiles.
  Softmax: global max/sum via DVE free-dim reduce + GPSIMD partition
    all-reduce; exp on ScalarE with -max bias; normalize by 1/sum.
  Small chain: conv3+relu, PE transposes for n^T, z, W^T = z^T @ w5^T.
  Phase 2: per 512-wide tile recompute x1 (k-major) from resident x,
    x_res = W @ x1 via W^T slices as stationary, residual add on DVE
    (exact f32 read of resident x), DMA out.

All matmuls run as float32r (full PE rate at N>=256) on f32 bits; the
residual add is exact f32 (f32r storage bitcast back to f32 — same bits).
Numerically safe: the softmax here is a near-argmax (gaps >> f32r
rounding) and |x_res| << |x|.
"""

import numpy as np

import concourse.bass as bass
import concourse.tile as tile
from concourse import bacc, mybir, bass_isa
from concourse.bass_utils import run_bass_kernel_spmd
from concourse.masks import make_identity

F32 = mybir.dt.float32
F32R = mybir.dt.float32r
AX = mybir.AxisListType
AL = mybir.AluOpType
AF = mybir.ActivationFunctionType

N_CORES = 8
C = 512
H = W_IMG = 96
HW = H * W_IMG          # 9216
CK = C // 128           # 4 chunks of channels
NBLK = 6                # x blocks along hw
BLK = HW // NBLK        # 1536
NSUB = HW // 128        # 72 phase-1 subtiles
SUB_PER_BLK = BLK // 128
NT = HW // 512          # 18 phase-2 tiles
T_PER_BLK = BLK // 512
C4 = C // 4             # 128
C2 = C // 2             # 256
KM = C4 + C2            # 384 = concat(x1T, x2T) free size


def _emit(ctx, tc, aps, use_bias):
    nc = tc.nc
    x_d = aps["x"]
    w12t_d = aps["w12t"]
    w3t_d = aps["w3t"]
    w4t_d = aps["w4t"]
    w5t_d = aps["w5t"]
    out_d = aps["out"]

    consts = ctx.enter_context(tc.tile_pool(name="consts", bufs=1))

    # ---- weights to SBUF (f32r: consumed only by matmuls) ----
    w12 = []
    for c in range(CK):
        t = consts.tile([128, KM], F32R, tag=f"w12_{c}")
        nc.sync.dma_start(out=t[:], in_=w12t_d[c * 128 : (c + 1) * 128, :])
        w12.append(t)
    w3t = consts.tile([128, 128], F32R, tag="w3t")
    nc.sync.dma_start(out=w3t[:], in_=w3t_d[:, :])
    w4t = []
    for q in range(2):
        t = consts.tile([128, C2], F32R, tag=f"w4t_{q}")
        nc.sync.dma_start(out=t[:], in_=w4t_d[q * 128 : (q + 1) * 128, :])
        w4t.append(t)
    w5t = []
    for q in range(2):
        t = consts.tile([128, C], F32R, tag=f"w5t_{q}")
        nc.sync.dma_start(out=t[:], in_=w5t_d[q * 128 : (q + 1) * 128, :])
        w5t.append(t)
    ident = consts.tile([128, 128], F32, tag="ident")
    make_identity(nc, ident[:])
    # f32r view of the identity for the phase-2 residual accumulate
    # (memset can't write f32r; DVE copy is a legal f32r-rounding producer)
    identr = consts.tile([128, 128], F32R, tag="identr")
    nc.vector.tensor_copy(identr[:], ident[:])

    bias_t = {}
    if use_bias:
        b12row_d = aps["b12row"]
        b1_d, b3_d, b4_d, b5_d = aps["b1c"], aps["b3c"], aps["b4c"], aps["b5c"]
        # [b1|b2] replicated across partitions, added to the hw-major tiles
        brow1 = consts.tile([1, KM], F32, tag="brow1")
        nc.sync.dma_start(out=brow1[:], in_=b12row_d[:, :])
        brow = consts.tile([128, KM], F32, tag="brow")
        nc.gpsimd.partition_broadcast(brow[:], brow1[:])
        bias_t["brow"] = brow
        b1 = consts.tile([128, 1], F32, tag="b1")
        nc.sync.dma_start(out=b1[:], in_=b1_d[:, :])
        bias_t["b1"] = b1
        b3 = consts.tile([128, 1], F32, tag="b3")
        nc.sync.dma_start(out=b3[:], in_=b3_d[:, :])
        bias_t["b3"] = b3
        b4 = []
        for q in range(2):
            t = consts.tile([128, 1], F32, tag=f"b4_{q}")
            nc.sync.dma_start(out=t[:], in_=b4_d[q * 128 : (q + 1) * 128, :])
            b4.append(t)
        bias_t["b4"] = b4
        b5 = []
        for oc in range(CK):
            t = consts.tile([128, 1], F32, tag=f"b5_{oc}")
            nc.sync.dma_start(out=t[:], in_=b5_d[oc * 128 : (oc + 1) * 128, :])
            b5.append(t)
        bias_t["b5"] = b5

    # ---- x resident in SBUF: 24 tiles [128, 1536] f32r ----
    # Block 0 is DMA'd in [128, 512] pieces (c-interleaved) so the first
    # compute subtiles become ready ~3x sooner; later blocks use one big
    # transfer each for bandwidth.
    xpool = ctx.enter_context(tc.tile_pool(name="x", bufs=1))
    xt = {}
    for b in range(NBLK):
        for c in range(CK):
            xt[(c, b)] = xpool.tile(
                [128, BLK], F32R, tag=f"x_{c}_{b}", name=f"x_{c}_{b}"
            )
    for p in range(BLK // 512):
        for c in range(CK):
            nc.sync.dma_start(
                out=xt[(c, 0)][:, p * 512 : (p + 1) * 512],
                in_=x_d[c * 128 : (c + 1) * 128, p * 512 : (p + 1) * 512],
            )
    for b in range(1, NBLK):
        for c in range(CK):
            nc.sync.dma_start(
                out=xt[(c, b)][:],
                in_=x_d[c * 128 : (c + 1) * 128, b * BLK : (b + 1) * BLK],
            )

    sm = ctx.enter_context(tc.tile_pool(name="sm", bufs=1))

    # ---- phase 1: x12T tiles + v accumulation ----
    with (
        tc.tile_pool(name="psA", bufs=3, space="PSUM") as psA,
        tc.tile_pool(name="vps", bufs=1, space="PSUM") as vps,
        tc.tile_pool(name="xtp", bufs=4) as xtp,
    ):
        v_ps = vps.tile([128, C2], F32, tag="v")

        def emit_v(s, xtile):
            nc.tensor.matmul(
                v_ps[:],
                xtile[:, 0:C4],
                xtile[:, C4:KM],
                start=(s == 0),
                stop=(s == NSUB - 1),
            )

        SKEW = 2
        pend = []
        for s in range(NSUB):
            b, off = divmod(s, SUB_PER_BLK)
            off *= 128
            ps = psA.tile([128, KM], F32, tag="xts")
            for c in range(CK):
                nc.tensor.matmul(
                    ps[:],
                    xt[(c, b)][:, off : off + 128],
                    w12[c][:],
                    start=(c == 0),
                    stop=(c == CK - 1),
                )
            xtile = xtp.tile([128, KM], F32R, tag="xt")
            if use_bias:
                nc.vector.tensor_tensor(
                    xtile[:], ps[:], bias_t["brow"][:], op=AL.add
                )
            else:
                nc.scalar.copy(xtile[:], ps[:])
            pend.append((s, xtile))
            if len(pend) > SKEW:
                emit_v(*pend.pop(0))
        while pend:
            emit_v(*pend.pop(0))

        # ---- softmax over all 32768 entries of v ----
        m1 = sm.tile([128, 1], F32, tag="m1")
        nc.vector.tensor_reduce(m1[:], v_ps[:], axis=AX.X, op=AL.max)
        mall = sm.tile([128, 1], F32, tag="mall")
        nc.gpsimd.partition_all_reduce(mall[:], m1[:], 128, bass_isa.ReduceOp.max)
        negm = sm.tile([128, 1], F32, tag="negm")
        nc.vector.tensor_scalar_mul(negm[:], mall[:], -1.0)
        e = sm.tile([128, C2], F32, tag="e")
        nc.scalar.activation(e[:], v_ps[:], AF.Exp, bias=negm[:], scale=1.0)

    s1 = sm.tile([128, 1], F32, tag="s1")
    nc.vector.tensor_reduce(s1[:], e[:], axis=AX.X, op=AL.add)
    sall = sm.tile([128, 1], F32, tag="sall")
    nc.gpsimd.partition_all_reduce(sall[:], s1[:], 128, bass_isa.ReduceOp.add)
    sinv = sm.tile([128, 1], F32, tag="sinv")
    nc.vector.reciprocal(sinv[:], sall[:])
    en = sm.tile([128, C2], F32R, tag="en")
    nc.vector.tensor_scalar_mul(en[:], e[:], sinv[:])

    # ---- small chain: conv3+relu, n^T, z, W^T ----
    wt = sm.tile([128, C], F32R, tag="wt")
    with tc.tile_pool(name="psB", bufs=2, space="PSUM") as psB:
        ps3 = psB.tile([128, C2], F32, tag="ps3")
        nc.tensor.matmul(ps3[:], w3t[:], en[:], start=True, stop=True)
        nsb = sm.tile([128, C2], F32, tag="nsb")
        b3s = bias_t["b3"][:] if use_bias else 0.0
        nc.vector.scalar_tensor_tensor(
            nsb[:], ps3[:], b3s, en[:].bitcast(F32), op0=AL.add, op1=AL.add
        )
        nc.vector.tensor_scalar_max(nsb[:], nsb[:], 0.0)

        nts = []
        for q in range(2):
            pT = psB.tile([128, 128], F32, tag="pT")
            nc.tensor.transpose(pT[:], nsb[:, q * 128 : (q + 1) * 128], ident[:])
            ntq = sm.tile([128, 128], F32R, tag=f"nt{q}")
            nc.scalar.copy(ntq[:], pT[:])
            nts.append(ntq)

        zs = []
        for mc in range(2):
            pz = psB.tile([128, 128], F32, tag="pz")
            for q in range(2):
                nc.tensor.matmul(
                    pz[:],
                    w4t[q][:, mc * 128 : (mc + 1) * 128],
                    nts[q][:],
                    start=(q == 0),
                    stop=(q == 1),
                )
            zq = sm.tile([128, 128], F32R, tag=f"z{mc}")
            if use_bias:
                nc.scalar.add(zq[:], pz[:], bias_t["b4"][mc][:])
            else:
                nc.scalar.copy(zq[:], pz[:])
            zs.append(zq)

        pW = psB.tile([128, C], F32, tag="pW")
        for mc in range(2):
            nc.tensor.matmul(
                pW[:], zs[mc][:], w5t[mc][:], start=(mc == 0), stop=(mc == 1)
            )
        nc.scalar.copy(wt[:], pW[:])

    # ---- phase 2: x1 recompute, x_res = W @ x1, residual add, DMA out ----
    # The residual add x + x_res runs on the PE: an f32r identity matmul
    # accumulates x into the same PSUM bank as x_res (f32r keeps 12
    # mantissa bits -> worst-case ~2.4e-4 relative on the passthrough,
    # well inside budget). PSUM->SBUF copies then alternate ACT/DVE.
    with (
        tc.tile_pool(name="psC", bufs=2, space="PSUM") as psC,
        tc.tile_pool(name="psD", bufs=5, space="PSUM") as psD,
        tc.tile_pool(name="x1p", bufs=4) as x1p,
        tc.tile_pool(name="outp", bufs=8) as outp,
    ):

        def emit_tail(t, x1tile):
            b, off = divmod(t, T_PER_BLK)
            off *= 512
            goff = t * 512
            for oc in range(CK):
                pr = psD.tile([128, 512], F32, tag="pr")
                nc.tensor.matmul(
                    pr[:],
                    wt[:, oc * 128 : (oc + 1) * 128],
                    x1tile[:],
                    start=True,
                    stop=False,
                )
                nc.tensor.matmul(
                    pr[:],
                    identr[:],
                    xt[(oc, b)][:, off : off + 512],
                    start=False,
                    stop=True,
                )
                ot = outp.tile([128, 512], F32, tag="ot")
                b5s = bias_t["b5"][oc][:] if use_bias else None
                if (t * CK + oc) % 2 == 0:
                    if b5s is not None:
                        nc.scalar.add(ot[:], pr[:], b5s)
                    else:
                        nc.scalar.copy(ot[:], pr[:])
                else:
                    if b5s is not None:
                        nc.vector.tensor_scalar_add(ot[:], pr[:], b5s)
                    else:
                        nc.vector.tensor_copy(ot[:], pr[:])
                nc.sync.dma_start(
                    out=out_d[oc * 128 : (oc + 1) * 128, goff : goff + 512],
                    in_=ot[:],
                )

        prev = None
        for t in range(NT):
            b, off = divmod(t, T_PER_BLK)
            off *= 512
            px1 = psC.tile([128, 512], F32, tag="px1")
            for c in range(CK):
                nc.tensor.matmul(
                    px1[:],
                    w12[c][:, 0:C4],
                    xt[(c, b)][:, off : off + 512],
                    start=(c == 0),
                    stop=(c == CK - 1),
                )
            x1tile = x1p.tile([128, 512], F32R, tag="x1t")
            if use_bias:
                nc.scalar.add(x1tile[:], px1[:], bias_t["b1"][:])
            else:
                nc.scalar.copy(x1tile[:], px1[:])
            if prev is not None:
                emit_tail(*prev)
            prev = (t, x1tile)
        emit_tail(*prev)


def _build(use_bias):
    nc = bacc.Bacc("TRN2", target_bir_lowering=False, debug=False, num_devices=N_CORES)
    aps = {
        "x": nc.dram_tensor("x", [C, HW], F32R, kind="ExternalInput").ap(),
        "w12t": nc.dram_tensor("w12t", [C, KM], F32R, kind="ExternalInput").ap(),
        "w3t": nc.dram_tensor("w3t", [C4, C4], F32R, kind="ExternalInput").ap(),
        "w4t": nc.dram_tensor("w4t", [C2, C2], F32R, kind="ExternalInput").ap(),
        "w5t": nc.dram_tensor("w5t", [C2, C], F32R, kind="ExternalInput").ap(),
        "out": nc.dram_tensor("out", [C, HW], F32, kind="ExternalOutput").ap(),
    }
    if use_bias:
        aps["b12row"] = nc.dram_tensor(
            "b12row", [1, KM], F32, kind="ExternalInput"
        ).ap()
        aps["b1c"] = nc.dram_tensor("b1c", [C4, 1], F32, kind="ExternalInput").ap()
        aps["b3c"] = nc.dram_tensor("b3c", [C4, 1], F32, kind="ExternalInput").ap()
        aps["b4c"] = nc.dram_tensor("b4c", [C2, 1], F32, kind="ExternalInput").ap()
        aps["b5c"] = nc.dram_tensor("b5c", [C, 1], F32, kind="ExternalInput").ap()

    from contextlib import ExitStack

    with tile.TileContext(nc) as tc:
        with ExitStack() as ctx:
            _emit(ctx, tc, aps, use_bias)
    nc.compile()
    return nc


_CACHE = {}


def _run(inputs, trace=False, **run_kwargs):
    x = np.ascontiguousarray(np.asarray(inputs["x"], dtype=np.float32))
    assert x.shape == (N_CORES, C, H, W_IMG), x.shape
    w1 = np.asarray(inputs["w1"], dtype=np.float32)
    w2 = np.asarray(inputs["w2"], dtype=np.float32)
    w3 = np.asarray(inputs["w3"], dtype=np.float32)
    w4 = np.asarray(inputs["w4"], dtype=np.float32)
    w5 = np.asarray(inputs["w5"], dtype=np.float32)
    b1 = np.asarray(inputs["b1"], dtype=np.float32)
    b2 = np.asarray(inputs["b2"], dtype=np.float32)
    b3 = np.asarray(inputs["b3"], dtype=np.float32)
    b4 = np.asarray(inputs["b4"], dtype=np.float32)
    b5 = np.asarray(inputs["b5"], dtype=np.float32)
    use_bias = bool(
        np.any(b1) or np.any(b2) or np.any(b3) or np.any(b4) or np.any(b5)
    )

    if use_bias not in _CACHE:
        _CACHE[use_bias] = _build(use_bias)
    nc = _CACHE[use_bias]

    w12t = np.ascontiguousarray(
        np.concatenate([w1.T, w2.T], axis=1), dtype=np.float32
    )  # [512, 384]
    w3t = np.ascontiguousarray(w3.T)
    w4t = np.ascontiguousarray(w4.T)
    w5t = np.ascontiguousarray(w5.T)

    shared = {"w12t": w12t, "w3t": w3t, "w4t": w4t, "w5t": w5t}
    if use_bias:
        shared["b12row"] = np.ascontiguousarray(
            np.concatenate([b1, b2])[None, :], dtype=np.float32
        )
        shared["b1c"] = np.ascontiguousarray(b1[:, None])
        shared["b3c"] = np.ascontiguousarray(b3[:, None])
        shared["b4c"] = np.ascontiguousarray(b4[:, None])
        shared["b5c"] = np.ascontiguousarray(b5[:, None])

    in_maps = [
        {"x": np.ascontiguousarray(x[b].reshape(C, HW)), **shared}
        for b in range(N_CORES)
    ]
    res = run_bass_kernel_spmd(
        nc, in_maps, core_ids=list(range(N_CORES)), trace=trace, **run_kwargs
    )
    out = np.stack(
        [res.results[b]["out"].reshape(C, H, W_IMG) for b in range(N_CORES)]
    ).astype(np.float32)
    return out, res


def kernel(**inputs):
    out, _ = _run(inputs, trace=False)
    return out
